# revision 12
# baseline (speedup 1.0000x reference)
"""Bass/Trainium2 kernel for nn_CustomBBoxLoss (v3: fp8 DoubleRow + box windows).

Reference computation:
    A1 = pred.sum(axis=(0,1));  A2 = (pred**2).sum(axis=(0,1))      # [H, W]
    s1[b] = sum of A1 over box b's region;  s2[b] likewise for A2
    per_box = (s2 - 2*cls*s1 + cls^2*cnt) / cnt;  loss = per_box.mean()

Each region sum is a bilinear form  s[b] = rowmask_b^T @ A @ colmask_b.
Structural facts exploited:
  * box_h <= 128, so a box's row mask touches <= 2 adjacent 128-row tiles.
    Sorting boxes by y makes the boxes relevant to any 512-row slab a
    CONTIGUOUS window of sorted indices (max span 87 < 128 here), so one
    <=128-wide stationary mask covers a whole core's rows and every data
    column streams through the PE exactly once.
  * fp8(e4m3) is ample for the 2e-2 tolerance (measured ~1e-3 end to end):
    pred streams from HBM as fp8 (1/4 the bytes) and the PE runs fp8
    DoubleRow matmuls contracting the two maps of a pair in one pass, so
    no pair-add elementwise work exists at all.

Per stream tile k=(rt,j) a combined SBUF tile holds [a | b | a^2 | b^2]
(1024 cols each; a,b = the two maps of pair j restricted to row tile rt).
The raw halves arrive as one contiguous DMA (alternating between the sync
and scalar HWDGE queues so two transfers are always in flight); squares
are computed by one fused ACT/DVE/GPSIMD op per tile (round-robin by
measured engine rates); four DoubleRow matmuls per tile accumulate
  psum[box, 0:1024]    += rmask^T (a + b)       (s1 partials)
  psum[box, 1024:2048] += rmask^T (a^2 + b^2)   (s2 partials)
into one 4-bank PSUM group over all 12 tiles.  Matmuls of the same row
tile share their stationary mask, so LDWEIGHTS is elided for all but the
first (ldweights=False).  The epilogue applies the column mask with two
fused multiply-accumulate DVE ops.

Sharding: 4x2 grid (512 rows x 1024 cols per core).  Masks are built
exactly on the host (integer compares) and DMA'd; row masks are staged
pre-replicated for DoubleRow's 2-deep K layout.  The host sums per-core
partials (the "all-reduce") and applies the closed-form per-box formula.
"""

import numpy as np
import ml_dtypes

F8 = ml_dtypes.float8_e4m3fn

H = W = 2048
B, C, N = 2, 3, 256
MAPS = B * C                      # 6
RB, CB = 4, 2                     # row-blocks x col-blocks = 8 cores
ROWS, COLS = H // RB, W // CB     # 512 x 1024 per core
P = 128                           # partitions
NRT = ROWS // P                   # 4 row tiles per core
NPAIR = MAPS // 2                 # 3 map pairs
NK = NRT * NPAIR                  # 12 streamed tiles per core
NBOX = 128                        # sorted-box window width per row slab

_CACHE = {}

# square-op engine per tile k.  ACT ~2.0us, DVE ~2.3us per fused
# [128,2048] op; GPSIMD is excluded: it shares an SBUF write port with
# the DVE, and concurrent squares halved both engines' throughput.
# k=11 is split between both engines so they finish together.
SQ_ENGINE = ["act", "dve", "act", "dve", "act", "dve",
             "act", "dve", "act", "dve", "act", "split"]


def _build_module():
    import concourse.bacc as bacc
    import concourse.mybir as mybir
    import concourse.tile as tile

    f32 = mybir.dt.float32
    f8 = mybir.dt.float8e4
    Alu = mybir.AluOpType
    DR = mybir.MatmulPerfMode.DoubleRow

    nc = bacc.Bacc("TRN2", target_bir_lowering=False, debug=False)

    # partition-major: per partition, NK tiles of [a | b] (2KB each)
    pred_part = nc.declare_dram_parameter("pred_part", [P, NK * 2048], f8, isOutput=False)
    # row masks per row tile, replicated for DoubleRow: [p, (t box)]
    rmt = nc.declare_dram_parameter("rmt", [NRT, P, 2 * NBOX], f8, isOutput=False)
    cm = nc.declare_dram_parameter("cm", [P, COLS], f8, isOutput=False)
    out_s = nc.declare_dram_parameter("out_s", [P, 2], f32, isOutput=True)

    with tile.TileContext(nc) as tc:
        with (
            tc.tile_pool(name="persist", bufs=1) as pp,
            tc.tile_pool(name="comb", bufs=6) as comb_pool,
            tc.tile_pool(name="psum", bufs=1, space="PSUM") as psum_pool,
        ):
            # ---- junk memset first on gpsimd (PE warm must not wait on
            # the mask DMA queue) ----
            junk = pp.tile([P, 512], f8, tag="junk", name="junk")
            nc.gpsimd.memset(junk[:], 0.0)

            # ---- mask DMAs on the gpsimd queue (keeps HWDGE queues clear) ----
            rm_t = []
            for rt in range(NRT):
                t = pp.tile([P, 2 * NBOX], f8, tag=f"rm{rt}", name=f"rm{rt}")
                nc.gpsimd.dma_start(t[:], rmt.ap()[rt])
                rm_t.append(t)
            cm_t = pp.tile([P, COLS], f8, tag="cm", name="cm")
            nc.gpsimd.dma_start(cm_t[:], cm.ap()[:])

            # ---- big stream: six "duo" tiles of 2 combs each; one DMA per
            # duo alternating across the sync/scalar HWDGE queues (separate
            # tiles keep the dependency tracking per-duo, and each queue has
            # only ~4 DMA semaphore slots) ----
            srcv = pred_part.ap()[:].rearrange("p (k c) -> p k c", k=NK)
            duos = []
            for i in range(NK // 2):
                duo = comb_pool.tile([P, 8192], f8, tag="duo", name="duo")
                dv = duo[:].rearrange("p (k c) -> p k c", k=2)
                q = nc.sync if i % 2 == 0 else nc.scalar
                q.dma_start(dv[:, :, 0:2048], srcv[:, 2 * i:2 * i + 2, :])
                duos.append(duo)
            combs = [duos[k // 2][:, (k % 2) * 4096:(k % 2 + 1) * 4096]
                     for k in range(NK)]

            # ---- PE pipeline warm ----
            ps = psum_pool.tile([P, 2048], f32, tag="ps", name="ps")
            for _ in range(8):
                nc.tensor.matmul(ps[:, 0:512], junk[:, :P], junk[:],
                                 start=True, stop=True)

            s_all = pp.tile([P, 2], f32, tag="s_all", name="s_all")

            # ---- squares: one fused op per tile, engine per SQ_ENGINE ----
            for k in range(NK):
                cb = combs[k]
                eng = SQ_ENGINE[k]
                if eng == "act":
                    nc.scalar.square(cb[:, 2048:4096], cb[:, 0:2048])
                elif eng == "dve":
                    nc.vector.tensor_mul(cb[:, 2048:4096],
                                         cb[:, 0:2048], cb[:, 0:2048])
                else:  # split between both engines
                    nc.scalar.square(cb[:, 2048:3072], cb[:, 0:1024])
                    nc.vector.tensor_mul(cb[:, 3072:4096],
                                         cb[:, 1024:2048], cb[:, 1024:2048])

            # ---- 4 DoubleRow matmuls per tile (k order == completion order) ----
            for i, k in enumerate(range(NK)):
                rt = k // NPAIR
                cb_tile = combs[k]
                lhsT = rm_t[rt][:].rearrange("p (t b) -> p t b", t=2)
                v4 = cb_tile[:].rearrange("p (t c) -> p t c", t=4)
                for q in range(2):          # 0: s1 from [a|b], 1: s2 from [sq]
                    rhs2 = v4[:, 2 * q:2 * q + 2, :]
                    for n in range(2):      # PSUM bank halves
                        nc.tensor.matmul(
                            ps[:, q * 1024 + n * 512:q * 1024 + (n + 1) * 512],
                            lhsT, rhs2[:, :, n * 512:(n + 1) * 512],
                            start=(i == 0), stop=(i == NK - 1), perf_mode=DR)

            # ---- epilogue: s[p] = sum_c psum[p, c] * cmask[p, c] ----
            scr = pp.tile([P, COLS], f32, tag="scr", name="scr")
            scr2 = pp.tile([P, COLS], f32, tag="scr2", name="scr2")
            nc.vector.scalar_tensor_tensor(
                out=scr[:], in0=ps[:, :COLS], scalar=1.0, in1=cm_t[:],
                op0=Alu.mult, op1=Alu.mult, accum_out=s_all[:, 0:1])
            nc.vector.scalar_tensor_tensor(
                out=scr2[:], in0=ps[:, COLS:], scalar=1.0, in1=cm_t[:],
                op0=Alu.mult, op1=Alu.mult, accum_out=s_all[:, 1:2])
            nc.sync.dma_start(out_s.ap()[:], s_all[:])

    _dedupe_ldweights(nc)
    nc.compile()
    return nc


def _dedupe_ldweights(nc):
    """Drop InstLdweights that reload the stationary already in the PE array.

    tile_legalize splits every matmul into InstLdweights + InstMatmult
    (ldweights=False); consecutive matmuls sharing a stationary then reload
    it pointlessly (~150ns of PE time each).  Remove waitless repeats.
    """
    for fn in nc.m.functions:
        for bb in fn.blocks:
            insts = list(bb.instructions)
            keep, removed = [], []
            last_sig = None
            for inst in insts:
                tn = type(inst).__name__
                if tn == "InstLdweights":
                    sig = (str(inst.ins[0]), str(inst.tile_size),
                           str(inst.tile_position), str(inst.perf_mode))
                    if sig == last_sig and not inst.has_wait():
                        removed.append(inst.name)
                        continue
                    last_sig = sig
                elif tn == "InstMatmult":
                    pass            # keeps the loaded stationary
                elif tn in ("InstEventSemaphore", "InstDrain", "InstNoOp"):
                    pass            # no effect on the PE array
                else:
                    last_sig = None
                keep.append(inst)
            if removed:
                bb.instructions = keep
                for inst in keep:
                    for nm in removed:
                        try:
                            inst.try_remove_dependency(nm)
                        except Exception:
                            pass


def _get_module():
    if "nc" not in _CACHE:
        _CACHE["nc"] = _build_module()
    return _CACHE["nc"]


def _plan_boxes(box_y, box_h):
    """Sort boxes by y; pick a 128-wide sorted window per row slab."""
    order = np.argsort(box_y, kind="stable")
    ys = box_y[order].astype(np.int64)
    hs = box_h[order].astype(np.int64)
    win = []
    for rb in range(RB):
        lo, hi = rb * ROWS, (rb + 1) * ROWS
        touch = np.nonzero((ys + hs > lo) & (ys < hi))[0]
        if len(touch) == 0:
            w0 = 0
        else:
            w0 = min(int(touch[0]), N - NBOX)
            assert int(touch[-1]) < w0 + NBOX, (
                f"slab {rb}: sorted-box window span {int(touch[-1]) - int(touch[0]) + 1}"
                f" exceeds {NBOX}")
        win.append(w0)
    return order, win


def _make_in_maps(pred, box_y, box_x, box_h, box_w, order, win):
    pred8 = pred.reshape(MAPS, H, W).astype(F8)
    ys = box_y[order].astype(np.int64)
    hs = box_h[order].astype(np.int64)
    xs = box_x[order].astype(np.int64)
    ws = box_w[order].astype(np.int64)

    in_maps = []
    for core in range(RB * CB):
        rb, cb = divmod(core, CB)
        slab = pred8[:, rb * ROWS:(rb + 1) * ROWS, cb * COLS:(cb + 1) * COLS]
        a = slab.reshape(NPAIR, 2, NRT, P, COLS)              # [j, t, rt, p, c]
        a = np.ascontiguousarray(a.transpose(3, 2, 0, 1, 4))  # [p, rt, j, t, c]

        w0 = win[rb]
        yw = ys[w0:w0 + NBOX]
        hw_ = hs[w0:w0 + NBOX]
        xw = xs[w0:w0 + NBOX] - cb * COLS
        ww = ws[w0:w0 + NBOX]

        # row masks: rm[rt, p, b] = yw[b] <= r < yw[b]+hw[b], r global row
        r = (rb * ROWS + np.arange(ROWS)).reshape(NRT, P, 1)
        rm = ((yw.reshape(1, 1, NBOX) <= r)
              & (r < (yw + hw_).reshape(1, 1, NBOX)))
        rmt_host = np.concatenate([rm, rm], axis=2).astype(F8)   # [rt, p, 2*NBOX]

        # col mask: cm[p, c] = xw[p] <= c < xw[p]+ww[p] (core-local cols)
        c = np.arange(COLS).reshape(1, COLS)
        cmh = ((xw.reshape(NBOX, 1) <= c)
               & (c < (xw + ww).reshape(NBOX, 1))).astype(F8)

        in_maps.append({
            "pred_part": a.reshape(P, NK * 2048),
            "rmt": np.ascontiguousarray(rmt_host),
            "cm": np.ascontiguousarray(cmh),
        })
    return in_maps


def _finalize(results, box_h, box_w, box_cls, order, win):
    s1 = np.zeros(N, np.float64)
    s2 = np.zeros(N, np.float64)
    for core, r in enumerate(results):
        rb = core // CB
        o = r["out_s"].astype(np.float64)          # [128, (s1, s2)]
        w0 = win[rb]
        s1[w0:w0 + NBOX] += o[:, 0]
        s2[w0:w0 + NBOX] += o[:, 1]
    hs = box_h[order].astype(np.float64)
    ws = box_w[order].astype(np.float64)
    cls = box_cls[order].astype(np.float64)
    cnt = float(MAPS) * hs * ws
    per_box = (s2 - 2.0 * cls * s1 + cls * cls * cnt) / cnt
    return np.asarray(per_box.mean(), dtype=np.float32)


def kernel(pred, box_y, box_x, box_h, box_w, box_cls, _bench=None):
    from concourse.bass_utils import run_bass_kernel_spmd

    pred = np.asarray(pred, dtype=np.float32)
    box_y = np.asarray(box_y, dtype=np.int32)
    box_x = np.asarray(box_x, dtype=np.int32)
    box_h = np.asarray(box_h, dtype=np.int32)
    box_w = np.asarray(box_w, dtype=np.int32)
    box_cls = np.asarray(box_cls, dtype=np.int32)

    nc = _get_module()
    order, win = _plan_boxes(box_y, box_h)
    in_maps = _make_in_maps(pred, box_y, box_x, box_h, box_w, order, win)
    kw = dict(_bench) if _bench else {}
    try:
        res = run_bass_kernel_spmd(nc, in_maps, core_ids=list(range(RB * CB)), **kw)
    except Exception:
        # transient NRT/device hiccups happen; one clean retry
        res = run_bass_kernel_spmd(nc, in_maps, core_ids=list(range(RB * CB)), **kw)
    if _bench is not None:
        _CACHE["last_results"] = res
    return _finalize(res.results, box_h, box_w, box_cls, order, win)
